# revision 32
# baseline (speedup 1.0000x reference)
"""Trainium2 Bass kernel for nn_ExchangeForecaster (MC-sampling MLP forecaster).

Data-parallel over batch: B=1024 rows -> 128 rows on each of 8 NeuronCores.
The Monte-Carlo z draws are fixed-key PRNG constants (independent of all
inputs), generated host-side with jax CPU threefry and shipped as inputs.

Math (per batch row b):
  ctx = pt[176:512]; loc = median(ctx); scale = max(mean|ctx-loc|, 1e-6)
  X = [z_s(30), ones(2), lags(8x336, robust-normed), diff(336), rstd(336),
       tf(4x336), ftf(120)]
  h1 = gelu_tanh(X @ W1 + b1); h2 = gelu_tanh(h1 @ W2 + b2); u = h2 @ W3 + b3
  pred = (z_s - u) * scale + loc

Kernel strategy:
  - base = X_static @ W1 computed once per row with the robust-norm affine
    *folded out* of the matmul (so the big matmul runs on raw lags, in
    parallel with the DVE median cascade).
  - per 8-sample group: psum1[h, g*128+b] = base-inject (matmul vs replicated
    identity) + W1[:30].T @ z^T; gelu on ACT; W2 matmul; gelu; per-sample
    W3 matmul back to [b, pred] layout; fused (z - u)*scale + loc on DVE.
"""

import os
import sys

sys.path.insert(0, "/opt/trn_rl_repo")

import numpy as np

CTX = 336
PRED = 30
NS = 100
LAGS = 8          # context + 7 lagged copies
HID = 128
B = 1024
TP = 512
NTF = 4
NCORES = 8
BC = B // NCORES  # 128 rows per core
T0 = TP - CTX     # 176

# feature-column layout of CHF = [lags | diff | rstd | tf | ftf]
F_LAG = LAGS * CTX            # 2688  (== 21*128, chunk-aligned)
F_DIFF = F_LAG                # 2688..3023
F_RSTD = F_LAG + CTX          # 3024..3359
F_TF = F_LAG + 2 * CTX        # 3360..4703
F_FT = F_TF + NTF * CTX       # 4704..4823
F_TOT = F_FT + NTF * PRED     # 4824
N_CHUNK = (F_TOT + 127) // 128  # 38
LAG_CHUNKS = F_LAG // 128       # 21

G = 8                          # samples per group
N_GROUPS = (NS + G - 1) // G   # 13 (12 full + 1 of 4)

_cache = {}


def _get_z():
    if "z" not in _cache:
        import jax

        with jax.default_device(jax.devices("cpu")[0]):
            z = jax.random.normal(jax.random.key(42), (NS, B, PRED), jax.numpy.float32)
            _cache["z"] = np.asarray(z)
    return _cache["z"]


def _build():
    if "nc" in _cache:
        return _cache["nc"]

    import concourse.bass as bass
    import concourse.mybir as mybir
    import concourse.tile as tile
    from concourse import bacc
    from concourse.masks import make_identity

    f32 = mybir.dt.float32
    f32r = mybir.dt.float32r
    AF = mybir.ActivationFunctionType
    OP = mybir.AluOpType

    nc = bacc.Bacc("TRN2", target_bir_lowering=False, debug=False)

    pt = nc.declare_dram_parameter("pt", [BC, TP], f32, isOutput=False)
    ptfc = nc.declare_dram_parameter("ptfc", [BC, CTX * NTF], f32, isOutput=False)
    ftf = nc.declare_dram_parameter("ftf", [BC, PRED * NTF], f32, isOutput=False)
    w1 = nc.declare_dram_parameter("w1", [30 + 2 + F_TOT, HID], f32, isOutput=False)
    w2 = nc.declare_dram_parameter("w2", [HID, HID], f32, isOutput=False)
    w3 = nc.declare_dram_parameter("w3", [HID, PRED], f32, isOutput=False)
    b1 = nc.declare_dram_parameter("b1", [HID], f32, isOutput=False)
    b2 = nc.declare_dram_parameter("b2", [HID], f32, isOutput=False)
    b3 = nc.declare_dram_parameter("b3", [PRED], f32, isOutput=False)
    # zt[d, s*128 + b] = z[s, b, d] (bf16);  zn[b, s*30 + p] = z[s, b, p]
    bf16 = mybir.dt.bfloat16
    zt = nc.declare_dram_parameter("zt", [PRED, NS * BC], bf16, isOutput=False)
    zn = nc.declare_dram_parameter("zn", [BC, NS * PRED], f32, isOutput=False)
    out = nc.declare_dram_parameter("out", [BC, NS * PRED], f32, isOutput=True)

    with tile.TileContext(nc) as tc:
        with (
            tc.tile_pool(name="const", bufs=1) as cpool,
            tc.tile_pool(name="feat", bufs=1) as fpool,
            tc.tile_pool(name="m8p", bufs=24) as m8pool,
        ):
            # ---------------- loads ----------------
            pt_sb = cpool.tile([BC, TP], f32)
            nc.sync.dma_start(out=pt_sb, in_=pt[:, :])
            ctx = pt_sb[:, T0:TP]

            ptf_sb = fpool.tile([BC, CTX * NTF], f32)
            nc.sync.dma_start(out=ptf_sb, in_=ptfc[:, :])
            ftf_sb = fpool.tile([BC, PRED * NTF], f32)
            nc.sync.dma_start(out=ftf_sb, in_=ftf[:, :])

            w1f32 = fpool.tile([128, N_CHUNK * 128], f32)
            nc.gpsimd.memset(w1f32[:, (N_CHUNK - 1) * 128 :], 0.0)
            # rows 32..4768 of W1, chunk-major: w1f[p, 128c + h] = W1[32+128c+p, h]
            nc.sync.dma_start(
                out=w1f32[:, : (N_CHUNK - 1) * 128].rearrange(
                    "p (c h) -> p c h", h=128
                ),
                in_=w1[32 : 32 + (N_CHUNK - 1) * 128, :].rearrange(
                    "(c p) h -> p c h", p=128
                ),
            )
            nrem = F_TOT - (N_CHUNK - 1) * 128  # 88
            nc.sync.dma_start(
                out=w1f32[:nrem, (N_CHUNK - 1) * 128 :],
                in_=w1[32 + (N_CHUNK - 1) * 128 : 32 + F_TOT, :],
            )
            w1f = cpool.tile([128, N_CHUNK * 128], bf16)
            nc.gpsimd.tensor_copy(out=w1f, in_=w1f32)
            w1z = fpool.tile([30, HID], f32)
            nc.sync.dma_start(out=w1z, in_=w1[0:30, :])
            w1zb = cpool.tile([30, HID], bf16)
            nc.scalar.copy(out=w1zb, in_=w1z)
            w1th = cpool.tile([2, HID], f32)
            nc.sync.dma_start(out=w1th, in_=w1[30:32, :])
            w2sb = fpool.tile([HID, HID], f32)
            nc.sync.dma_start(out=w2sb, in_=w2[:, :])
            w2b = cpool.tile([HID, HID], bf16)
            nc.scalar.copy(out=w2b, in_=w2sb)
            w3sb = fpool.tile([HID, PRED], f32)
            nc.sync.dma_start(out=w3sb, in_=w3[:, :])
            w3n = cpool.tile([HID, PRED], f32)
            nc.scalar.mul(out=w3n, in_=w3sb, mul=-1.0)
            b1c = cpool.tile([HID, 1], f32)
            nc.sync.dma_start(out=b1c, in_=b1[:, None])
            b2c = cpool.tile([HID, 1], f32)
            nc.sync.dma_start(out=b2c, in_=b2[:, None])
            # -b3 tiled x8, padded to 256 cols (>=256 free dim -> full-rate f32r)
            b3r8 = fpool.tile([1, 256], f32)
            nc.gpsimd.memset(b3r8[:, G * PRED :], 0.0)
            for k in range(G):
                nc.sync.dma_start(out=b3r8[0:1, k * PRED : (k + 1) * PRED], in_=b3[None, :])
            mb3r8 = cpool.tile([1, 256], f32)
            nc.scalar.mul(out=mb3r8, in_=b3r8, mul=-1.0)

            zn_sb = cpool.tile([BC, NS * PRED], f32)
            nc.sync.dma_start(out=zn_sb, in_=zn[:, :])

            I128 = cpool.tile([128, 128], f32)
            make_identity(nc, I128)
            i8 = cpool.tile([128, 4 * 128], bf16)
            for k in range(4):
                nc.gpsimd.tensor_copy(out=i8[:, k * 128 : (k + 1) * 128], in_=I128)
            ones_col = cpool.tile([128, 1], bf16)
            nc.gpsimd.memset(ones_col, 1.0)
            ones_row = cpool.tile([1, 128], f32)
            nc.gpsimd.memset(ones_row, 1.0)
            ones2 = cpool.tile([2, 128], f32)
            nc.gpsimd.memset(ones2, 1.0)

            # ---------------- CHF feature build (gpsimd mostly, bf16) ----------
            CHF = fpool.tile([BC, F_TOT], bf16)
            # lag channels: raw slices of pt (robust-norm folded into base later)
            for c in range(LAGS):
                nc.gpsimd.tensor_copy(
                    out=CHF[:, c * CTX : (c + 1) * CTX],
                    in_=pt_sb[:, T0 - c : TP - c],
                )
            # diff channel: build raw in dtmp, normalize into CHF via ACT scale
            dtmp = fpool.tile([BC, CTX], f32)
            nc.gpsimd.memset(dtmp[:, 0:1], 0.0)
            nc.vector.tensor_tensor(
                out=dtmp[:, 1:CTX], in0=ctx[:, 1:], in1=ctx[:, :-1], op=OP.subtract
            )
            dabs = fpool.tile([BC, CTX], f32)
            dsum = fpool.tile([BC, 1], f32)
            nc.scalar.activation(out=dabs, in_=dtmp, func=AF.Abs, accum_out=dsum)
            dden = fpool.tile([BC, 1], f32)
            nc.vector.tensor_scalar(
                out=dden, in0=dsum, scalar1=1.0 / CTX, scalar2=1e-6,
                op0=OP.mult, op1=OP.max,
            )
            drec = fpool.tile([BC, 1], f32)
            nc.vector.reciprocal(out=drec, in_=dden)
            nc.scalar.activation(
                out=CHF[:, F_DIFF : F_DIFF + CTX], in_=dtmp, func=AF.Copy, scale=drec
            )

            # rolling std channel (mirrors reference cumsum formulation)
            ctx2 = fpool.tile([BC, CTX], f32)
            nc.vector.tensor_tensor(out=ctx2, in0=ctx, in1=ctx, op=OP.mult)
            cs = fpool.tile([BC, CTX], f32)
            nc.vector.tensor_tensor_scan(
                out=cs, data0=ctx, data1=ctx, initial=0.0, op0=OP.add, op1=OP.bypass
            )
            cs2 = fpool.tile([BC, CTX], f32)
            nc.vector.tensor_tensor_scan(
                out=cs2, data0=ctx2, data1=ctx2, initial=0.0, op0=OP.add, op1=OP.bypass
            )
            s1 = fpool.tile([BC, CTX], f32)
            s2 = fpool.tile([BC, CTX], f32)
            for (st, src) in ((s1, cs), (s2, cs2)):
                nc.gpsimd.tensor_copy(out=st[:, 0:5], in_=src[:, 0:5])
                nc.vector.tensor_tensor(
                    out=st[:, 5:CTX], in0=src[:, 5:CTX], in1=src[:, 0 : CTX - 5],
                    op=OP.subtract,
                )
            # t = s1*s1; var = (s2 - t/n) / (n-1); n: col2->3, col3->4, col>=4->5
            tsq = fpool.tile([BC, CTX], f32)
            nc.vector.tensor_tensor(out=tsq, in0=s1, in1=s1, op=OP.mult)
            var = fpool.tile([BC, CTX], f32)
            for lo, hi, ninv in ((2, 3, -1.0 / 3), (3, 4, -0.25), (4, CTX, -0.2)):
                nc.vector.scalar_tensor_tensor(
                    out=var[:, lo:hi], in0=tsq[:, lo:hi], scalar=ninv,
                    in1=s2[:, lo:hi], op0=OP.mult, op1=OP.add,
                )
            for lo, hi, dinv in ((2, 3, 0.5), (3, 4, 1.0 / 3), (4, CTX, 0.25)):
                nc.vector.tensor_scalar(
                    out=var[:, lo:hi], in0=var[:, lo:hi], scalar1=dinv, scalar2=1e-12,
                    op0=OP.mult, op1=OP.max,
                )
            rst_raw = fpool.tile([BC, CTX], f32)
            nc.gpsimd.memset(rst_raw[:, 0:2], 0.0)
            nc.scalar.activation(
                out=rst_raw[:, 2:CTX], in_=var[:, 2:CTX], func=AF.Sqrt
            )
            rabs = fpool.tile([BC, CTX], f32)
            rsum = fpool.tile([BC, 1], f32)
            nc.scalar.activation(out=rabs, in_=rst_raw, func=AF.Abs, accum_out=rsum)
            rden = fpool.tile([BC, 1], f32)
            nc.vector.tensor_scalar(
                out=rden, in0=rsum, scalar1=1.0 / CTX, scalar2=1e-6,
                op0=OP.mult, op1=OP.max,
            )
            rrec = fpool.tile([BC, 1], f32)
            nc.vector.reciprocal(out=rrec, in_=rden)
            nc.scalar.activation(
                out=CHF[:, F_RSTD : F_RSTD + CTX], in_=rst_raw, func=AF.Copy, scale=rrec
            )

            # time-feature channels: CHF[:, F_TF + 336*f + t] = ptfc[b, t*4 + f]
            for f in range(NTF):
                nc.gpsimd.tensor_copy(
                    out=CHF[:, F_TF + f * CTX : F_TF + (f + 1) * CTX],
                    in_=ptf_sb.rearrange("b (t f) -> b t f", f=NTF)[:, :, f],
                )
            # future-time-feature block: CHF[:, F_FT + 30*f + p] = ftf[b, p*4 + f]
            for f in range(NTF):
                nc.gpsimd.tensor_copy(
                    out=CHF[:, F_FT + f * PRED : F_FT + (f + 1) * PRED],
                    in_=ftf_sb.rearrange("b (p f) -> b p f", f=NTF)[:, :, f],
                )

            # ---------------- median via max8/match_replace cascade (DVE) ------
            mwork = fpool.tile([BC, CTX], f32)
            nc.vector.tensor_copy(out=mwork, in_=ctx)
            n_rounds = (CTX // 2) // 8  # 21 -> extracts top 168
            m8_prev = None
            for r in range(n_rounds):
                m8 = m8pool.tile([BC, 8], f32, tag="m8")
                nc.vector.max(out=m8, in_=mwork)
                nc.vector.match_replace(
                    out=mwork, in_to_replace=m8, in_values=mwork, imm_value=-1e30
                )
                m8_prev = m8
            m8_last = m8pool.tile([BC, 8], f32, tag="m8")
            nc.vector.max(out=m8_last, in_=mwork)
            a168 = m8_prev[:, 7:8]   # 168th largest == a[168] (0-idx ascending)
            a167 = m8_last[:, 0:1]   # 169th largest == a[167]
            # jnp.median linear interp: a167 + 0.5*(a168 - a167)
            dmed = fpool.tile([BC, 1], f32)
            nc.vector.tensor_tensor(out=dmed, in0=a168, in1=a167, op=OP.subtract)
            loc = fpool.tile([BC, 1], f32)
            nc.vector.scalar_tensor_tensor(
                out=loc, in0=dmed, scalar=0.5, in1=a167, op0=OP.mult, op1=OP.add
            )
            mloc = fpool.tile([BC, 1], f32)
            nc.vector.tensor_scalar(out=mloc, in0=loc, scalar1=-1.0, scalar2=None, op0=OP.mult)
            sabs = fpool.tile([BC, CTX], f32)
            ssum = fpool.tile([BC, 1], f32)
            nc.scalar.activation(
                out=sabs, in_=ctx, func=AF.Abs, bias=mloc, scale=1.0, accum_out=ssum
            )
            scale = fpool.tile([BC, 1], f32)
            nc.vector.tensor_scalar(
                out=scale, in0=ssum, scalar1=1.0 / CTX, scalar2=1e-6,
                op0=OP.mult, op1=OP.max,
            )
            rs = fpool.tile([BC, 1], f32)
            nc.vector.reciprocal(out=rs, in_=scale)

            # ---------------- transpose CHF -> xT, base matmuls ----------------
            # bf16 identity for bf16 transposes
            Ib = i8[:, 0:128]
            xT = cpool.tile([128, N_CHUNK * 128], bf16)
            nc.gpsimd.memset(xT[:, (N_CHUNK - 1) * 128 :], 0.0)
            with tc.tile_pool(name="tps", bufs=3, space="PSUM") as tpool:
                # 9 groups of 4 chunks share one psum bank + one ACT copy
                for j0 in range(0, 36, 4):
                    tps = tpool.tile([128, 512], bf16, tag="tr")
                    for j in range(j0, j0 + 4):
                        nc.tensor.transpose(
                            out=tps[:, (j - j0) * 128 : (j - j0 + 1) * 128],
                            in_=CHF[:, j * 128 : (j + 1) * 128], identity=Ib,
                        )
                    nc.scalar.copy(
                        out=xT[:, j0 * 128 : (j0 + 4) * 128], in_=tps
                    )
                # chunks 36 (full) and 37 (88 rows) individually
                for j in (36, 37):
                    F = 128 if j < N_CHUNK - 1 else nrem
                    tps1 = tpool.tile([128, 512], bf16, tag="tr")
                    nc.tensor.transpose(
                        out=tps1[:F, 0:128],
                        in_=CHF[:, j * 128 : j * 128 + F], identity=Ib,
                    )
                    nc.scalar.copy(
                        out=xT[:F, j * 128 : (j + 1) * 128], in_=tps1[:F, 0:128]
                    )

            with tc.tile_pool(name="bps", bufs=1, space="PSUM") as bpool:
                psum_lag = bpool.tile([128, 128], f32)
                psum_rest = bpool.tile([128, 128], f32)
                psum_q = bpool.tile([1, 128], f32)
                for j in range(LAG_CHUNKS):
                    sl = slice(j * 128, (j + 1) * 128)
                    nc.tensor.matmul(
                        psum_lag, lhsT=xT[:, sl], rhs=w1f[:, sl],
                        start=(j == 0), stop=(j == LAG_CHUNKS - 1),
                    )
                    nc.tensor.matmul(
                        psum_q, lhsT=ones_col, rhs=w1f[:, sl],
                        start=(j == 0), stop=(j == LAG_CHUNKS - 1),
                    )
                for j in range(LAG_CHUNKS, N_CHUNK):
                    sl = slice(j * 128, (j + 1) * 128)
                    nc.tensor.matmul(
                        psum_rest, lhsT=xT[:, sl], rhs=w1f[:, sl],
                        start=(j == LAG_CHUNKS), stop=False,
                    )
                nc.tensor.matmul(psum_rest, lhsT=ones2, rhs=w1th, start=False, stop=True)

                q_row = fpool.tile([1, 128], f32)
                nc.vector.tensor_copy(out=q_row, in_=psum_q)
                psum_qB = bpool.tile([128, 128], f32)
                nc.tensor.matmul(psum_qB, lhsT=ones_row, rhs=q_row, start=True, stop=True)

                mlr = fpool.tile([BC, 1], f32)
                nc.vector.tensor_scalar(
                    out=mlr, in0=loc, scalar1=rs, scalar2=-1.0, op0=OP.mult, op1=OP.mult
                )
                t1 = fpool.tile([128, 128], f32)
                nc.vector.tensor_scalar(out=t1, in0=psum_lag, scalar1=rs, scalar2=None, op0=OP.mult)
                t2 = fpool.tile([128, 128], f32)
                nc.vector.scalar_tensor_tensor(
                    out=t2, in0=psum_qB, scalar=mlr, in1=t1, op0=OP.mult, op1=OP.add
                )
                baseN = fpool.tile([128, 128], f32)
                nc.vector.tensor_tensor(out=baseN, in0=t2, in1=psum_rest, op=OP.add)
                basebf = cpool.tile([128, 128], bf16)
                nc.scalar.copy(out=basebf, in_=baseN)

            # ---------------- sampling loop ----------------
            gelu = AF.Gelu_apprx_tanh
            with (
                tc.tile_pool(name="zp", bufs=2) as zpool,
                tc.tile_pool(name="hp", bufs=2) as hpool,
                tc.tile_pool(name="ap", bufs=2) as apool,
                tc.tile_pool(name="op", bufs=2) as opool,
                tc.tile_pool(name="sp", bufs=1, space="PSUM") as spool,
            ):
                for g in range(N_GROUPS):
                    Gg = min(G, NS - g * G)
                    W = Gg * 128
                    P = Gg * PRED
                    zt_g = zpool.tile([PRED, G * 128], bf16, tag="zt")
                    nc.sync.dma_start(
                        out=zt_g[:, :W], in_=zt[:, g * G * 128 : g * G * 128 + W]
                    )
                    # A = z*scale + loc  (b3 handled by rank-1 inject into psum_u)
                    A_g = apool.tile([BC, G * PRED], f32, tag="A")
                    nc.vector.tensor_scalar(
                        out=A_g[:, :P],
                        in0=zn_sb[:, g * G * PRED : g * G * PRED + P],
                        scalar1=scale, scalar2=loc, op0=OP.mult, op1=OP.add,
                    )
                    psum1 = spool.tile([128, G * 128], f32, tag="p1", bufs=2)
                    for k in range(W // 512):
                        sl = slice(k * 512, (k + 1) * 512)
                        nc.tensor.matmul(
                            psum1[:, sl], lhsT=basebf, rhs=i8, start=True, stop=False,
                        )
                        nc.tensor.matmul(
                            psum1[:, sl], lhsT=w1zb, rhs=zt_g[:, sl],
                            start=False, stop=True,
                        )
                    h1T = hpool.tile([128, G * 128], bf16, tag="h1")
                    nc.scalar.activation(out=h1T[:, :W], in_=psum1[:, :W], func=gelu, bias=b1c)
                    psum2 = spool.tile([128, G * 128], f32, tag="p2", bufs=2)
                    for k in range(W // 512):
                        sl = slice(k * 512, (k + 1) * 512)
                        nc.tensor.matmul(
                            psum2[:, sl], lhsT=w2b, rhs=h1T[:, sl], start=True, stop=True,
                            skip_group_check=True,
                        )
                    h2T = hpool.tile([128, G * 128], f32, tag="h2")
                    nc.scalar.activation(out=h2T[:, :W], in_=psum2[:, :W], func=gelu, bias=b2c)
                    # L3 reuses psum2's bank (its L2 contents are dead after gelu2)
                    psum_u = psum2[:, 0 : G * PRED]
                    # rank-1 inject of -b3 across all sample blocks, then -h2@W3
                    nc.tensor.matmul(
                        psum_u[:, :240], lhsT=ones_row, rhs=mb3r8[:, :240],
                        start=True, stop=False, skip_group_check=True,
                    )
                    for gg in range(Gg):
                        nc.tensor.matmul(
                            psum_u[:, gg * PRED : (gg + 1) * PRED],
                            lhsT=h2T[:, gg * 128 : (gg + 1) * 128],
                            rhs=w3n, start=False, stop=True, skip_group_check=True,
                        )
                    ob = opool.tile([BC, G * PRED], f32, tag="ob")
                    nc.vector.scalar_tensor_tensor(
                        out=ob[:, :P], in0=psum_u[:, :P], scalar=scale, in1=A_g[:, :P],
                        op0=OP.mult, op1=OP.add,
                    )
                    nc.sync.dma_start(
                        out=out[:, g * G * PRED : g * G * PRED + P], in_=ob[:, :P]
                    )

    nc.compile()
    _cache["nc"] = nc
    return nc


def _in_maps(inputs):
    z = _get_z()  # [NS, B, PRED]
    pt_full = np.ascontiguousarray(inputs["past_target"], np.float32)
    ptf_full = np.ascontiguousarray(inputs["past_time_feat"], np.float32)
    ftf_full = np.ascontiguousarray(inputs["future_time_feat"], np.float32)
    w1 = np.ascontiguousarray(inputs["W1"], np.float32)
    w2 = np.ascontiguousarray(inputs["W2"], np.float32)
    w3 = np.ascontiguousarray(inputs["W3"], np.float32)
    b1 = np.ascontiguousarray(inputs["b1"], np.float32)
    b2 = np.ascontiguousarray(inputs["b2"], np.float32)
    b3 = np.ascontiguousarray(inputs["b3"], np.float32)

    maps = []
    for i in range(NCORES):
        sl = slice(i * BC, (i + 1) * BC)
        import ml_dtypes

        zc = z[:, sl, :]  # [NS, BC, PRED]
        # zt[d, s*128+b] = z[s,b,d]  (bf16 for the full-rate W1z matmul)
        ztc = np.ascontiguousarray(
            zc.transpose(2, 0, 1).reshape(PRED, NS * BC)
        ).astype(ml_dtypes.bfloat16)
        znc = np.ascontiguousarray(
            zc.transpose(1, 0, 2).reshape(BC, NS * PRED), np.float32
        )
        maps.append(
            {
                "pt": pt_full[sl],
                "ptfc": np.ascontiguousarray(
                    ptf_full[sl, T0:TP, :].reshape(BC, CTX * NTF)
                ),
                "ftf": np.ascontiguousarray(ftf_full[sl].reshape(BC, PRED * NTF)),
                "w1": w1, "w2": w2, "w3": w3, "b1": b1, "b2": b2, "b3": b3,
                "zt": ztc, "zn": znc,
            }
        )
    return maps


def kernel(**inputs):
    nc = _build()
    from concourse.bass_utils import run_bass_kernel_spmd

    res = run_bass_kernel_spmd(nc, _in_maps(inputs), core_ids=list(range(NCORES)))
    _cache["last_result"] = res
    out = np.concatenate(
        [r["out"].reshape(BC, NS, PRED) for r in res.results], axis=0
    )
    return out


if __name__ == "__main__":
    _build()
    print("build ok")


# revision 34
# speedup vs baseline: 289.3795x; 289.3795x over previous
"""Trainium2 Bass kernel for nn_ExchangeForecaster (MC-sampling MLP forecaster).

Data-parallel over batch: B=1024 rows -> 128 rows on each of 8 NeuronCores.
The Monte-Carlo z draws are fixed-key PRNG constants (independent of all
inputs), generated host-side with jax CPU threefry and shipped as inputs.

Math (per batch row b):
  ctx = pt[176:512]; loc = median(ctx); scale = max(mean|ctx-loc|, 1e-6)
  X = [z_s(30), ones(2), lags(8x336, robust-normed), diff(336), rstd(336),
       tf(4x336), ftf(120)]
  h1 = gelu_tanh(X @ W1 + b1); h2 = gelu_tanh(h1 @ W2 + b2); u = h2 @ W3 + b3
  pred = (z_s - u) * scale + loc

Kernel strategy:
  - base = X_static @ W1 computed once per row with the robust-norm affine
    *folded out* of the matmul (so the big matmul runs on raw lags, in
    parallel with the DVE median cascade: exact order-stats 168/169 via
    21 rounds of max8 + match_replace).
  - static features (CHF), their PE transposes (xT) and the W1 body are
    bf16 (full-rate PE, half-size copies); the z-sampling path, robust-norm
    statistics, layer-3 and the final affine stay fp32. Verified on HW:
    L2 relative error ~1.2e-4 vs the fp32 reference.
  - per 8-sample group: psum1[h, g*128+b] = base-inject (bf16 matmul vs
    replicated identity) + W1[:30].T @ z^T (bf16); gelu_tanh on ACT
    (bias=b1 fused); W2 matmul (bf16); gelu_tanh (bias=b2); -b3 rank-1
    inject + per-sample -h2@W3 into the dead psum2 bank; one fused DVE
    scalar_tensor_tensor writes (z - u)*scale + loc straight to the
    output tile. PSUM: psum1 2 banks x2 bufs + psum2 2 banks x2 bufs.
"""

import os
import sys

sys.path.insert(0, "/opt/trn_rl_repo")

import numpy as np

CTX = 336
PRED = 30
NS = 100
LAGS = 8          # context + 7 lagged copies
HID = 128
B = 1024
TP = 512
NTF = 4
NCORES = 8
BC = B // NCORES  # 128 rows per core
T0 = TP - CTX     # 176

# feature-column layout of CHF = [lags | diff | rstd | tf | ftf]
F_LAG = LAGS * CTX            # 2688  (== 21*128, chunk-aligned)
F_DIFF = F_LAG                # 2688..3023
F_RSTD = F_LAG + CTX          # 3024..3359
F_TF = F_LAG + 2 * CTX        # 3360..4703
F_FT = F_TF + NTF * CTX       # 4704..4823
F_TOT = F_FT + NTF * PRED     # 4824
N_CHUNK = (F_TOT + 127) // 128  # 38
LAG_CHUNKS = F_LAG // 128       # 21

G = 8                          # samples per group
N_GROUPS = (NS + G - 1) // G   # 13 (12 full + 1 of 4)

_cache = {}


def _get_z():
    if "z" not in _cache:
        import jax

        with jax.default_device(jax.devices("cpu")[0]):
            z = jax.random.normal(jax.random.key(42), (NS, B, PRED), jax.numpy.float32)
            _cache["z"] = np.asarray(z)
    return _cache["z"]


def _build():
    if "nc" in _cache:
        return _cache["nc"]

    import concourse.bass as bass
    import concourse.mybir as mybir
    import concourse.tile as tile
    from concourse import bacc
    from concourse.masks import make_identity

    f32 = mybir.dt.float32
    AF = mybir.ActivationFunctionType
    OP = mybir.AluOpType

    nc = bacc.Bacc("TRN2", target_bir_lowering=False, debug=False)

    pt = nc.declare_dram_parameter("pt", [BC, TP], f32, isOutput=False)
    ptfc = nc.declare_dram_parameter("ptfc", [BC, CTX * NTF], f32, isOutput=False)
    ftf = nc.declare_dram_parameter("ftf", [BC, PRED * NTF], f32, isOutput=False)
    w1 = nc.declare_dram_parameter("w1", [30 + 2 + F_TOT, HID], f32, isOutput=False)
    w2 = nc.declare_dram_parameter("w2", [HID, HID], f32, isOutput=False)
    w3 = nc.declare_dram_parameter("w3", [HID, PRED], f32, isOutput=False)
    b1 = nc.declare_dram_parameter("b1", [HID], f32, isOutput=False)
    b2 = nc.declare_dram_parameter("b2", [HID], f32, isOutput=False)
    b3 = nc.declare_dram_parameter("b3", [PRED], f32, isOutput=False)
    # zt[d, s*128 + b] = z[s, b, d] (bf16);  zn[b, s*30 + p] = z[s, b, p]
    bf16 = mybir.dt.bfloat16
    zt = nc.declare_dram_parameter("zt", [PRED, NS * BC], bf16, isOutput=False)
    zn = nc.declare_dram_parameter("zn", [BC, NS * PRED], f32, isOutput=False)
    out = nc.declare_dram_parameter("out", [BC, NS * PRED], f32, isOutput=True)

    with tile.TileContext(nc) as tc:
        with (
            tc.tile_pool(name="const", bufs=1) as cpool,
            tc.tile_pool(name="feat", bufs=1) as fpool,
            tc.tile_pool(name="m8p", bufs=24) as m8pool,
        ):
            # ---------------- loads ----------------
            pt_sb = cpool.tile([BC, TP], f32)
            nc.sync.dma_start(out=pt_sb, in_=pt[:, :])
            ctx = pt_sb[:, T0:TP]

            ptf_sb = fpool.tile([BC, CTX * NTF], f32)
            nc.sync.dma_start(out=ptf_sb, in_=ptfc[:, :])
            ftf_sb = fpool.tile([BC, PRED * NTF], f32)
            nc.sync.dma_start(out=ftf_sb, in_=ftf[:, :])

            w1f32 = fpool.tile([128, N_CHUNK * 128], f32)
            nc.gpsimd.memset(w1f32[:, (N_CHUNK - 1) * 128 :], 0.0)
            # rows 32..4768 of W1, chunk-major: w1f[p, 128c + h] = W1[32+128c+p, h]
            nc.sync.dma_start(
                out=w1f32[:, : (N_CHUNK - 1) * 128].rearrange(
                    "p (c h) -> p c h", h=128
                ),
                in_=w1[32 : 32 + (N_CHUNK - 1) * 128, :].rearrange(
                    "(c p) h -> p c h", p=128
                ),
            )
            nrem = F_TOT - (N_CHUNK - 1) * 128  # 88
            nc.sync.dma_start(
                out=w1f32[:nrem, (N_CHUNK - 1) * 128 :],
                in_=w1[32 + (N_CHUNK - 1) * 128 : 32 + F_TOT, :],
            )
            w1f = cpool.tile([128, N_CHUNK * 128], bf16)
            nc.gpsimd.tensor_copy(out=w1f, in_=w1f32)
            w1z = fpool.tile([30, HID], f32)
            nc.sync.dma_start(out=w1z, in_=w1[0:30, :])
            w1zb = cpool.tile([30, HID], bf16)
            nc.scalar.copy(out=w1zb, in_=w1z)
            w1th = cpool.tile([2, HID], f32)
            nc.sync.dma_start(out=w1th, in_=w1[30:32, :])
            w2sb = fpool.tile([HID, HID], f32)
            nc.sync.dma_start(out=w2sb, in_=w2[:, :])
            w2b = cpool.tile([HID, HID], bf16)
            nc.scalar.copy(out=w2b, in_=w2sb)
            w3sb = fpool.tile([HID, PRED], f32)
            nc.sync.dma_start(out=w3sb, in_=w3[:, :])
            w3n = cpool.tile([HID, PRED], f32)
            nc.scalar.mul(out=w3n, in_=w3sb, mul=-1.0)
            b1c = cpool.tile([HID, 1], f32)
            nc.sync.dma_start(out=b1c, in_=b1[:, None])
            b2c = cpool.tile([HID, 1], f32)
            nc.sync.dma_start(out=b2c, in_=b2[:, None])
            # -b3 tiled x8, padded to 256 cols (>=256 free dim -> full-rate f32r)
            b3r8 = fpool.tile([1, 256], f32)
            nc.gpsimd.memset(b3r8[:, G * PRED :], 0.0)
            for k in range(G):
                nc.sync.dma_start(out=b3r8[0:1, k * PRED : (k + 1) * PRED], in_=b3[None, :])
            mb3r8 = cpool.tile([1, 256], f32)
            nc.scalar.mul(out=mb3r8, in_=b3r8, mul=-1.0)

            zn_sb = cpool.tile([BC, NS * PRED], f32)
            nc.sync.dma_start(out=zn_sb, in_=zn[:, :])

            I128 = cpool.tile([128, 128], f32)
            make_identity(nc, I128)
            i8 = cpool.tile([128, 4 * 128], bf16)
            for k in range(4):
                nc.gpsimd.tensor_copy(out=i8[:, k * 128 : (k + 1) * 128], in_=I128)
            ones_col = cpool.tile([128, 1], bf16)
            nc.gpsimd.memset(ones_col, 1.0)
            ones_row = cpool.tile([1, 128], f32)
            nc.gpsimd.memset(ones_row, 1.0)
            ones2 = cpool.tile([2, 128], f32)
            nc.gpsimd.memset(ones2, 1.0)

            # ---------------- CHF feature build (gpsimd mostly, bf16) ----------
            CHF = fpool.tile([BC, F_TOT], bf16)
            # lag channels: raw slices of pt (robust-norm folded into base later)
            for c in range(LAGS):
                nc.gpsimd.tensor_copy(
                    out=CHF[:, c * CTX : (c + 1) * CTX],
                    in_=pt_sb[:, T0 - c : TP - c],
                )
            # diff channel: build raw in dtmp, normalize into CHF via ACT scale
            dtmp = fpool.tile([BC, CTX], f32)
            nc.gpsimd.memset(dtmp[:, 0:1], 0.0)
            nc.vector.tensor_tensor(
                out=dtmp[:, 1:CTX], in0=ctx[:, 1:], in1=ctx[:, :-1], op=OP.subtract
            )
            dabs = fpool.tile([BC, CTX], f32)
            dsum = fpool.tile([BC, 1], f32)
            nc.scalar.activation(out=dabs, in_=dtmp, func=AF.Abs, accum_out=dsum)
            dden = fpool.tile([BC, 1], f32)
            nc.vector.tensor_scalar(
                out=dden, in0=dsum, scalar1=1.0 / CTX, scalar2=1e-6,
                op0=OP.mult, op1=OP.max,
            )
            drec = fpool.tile([BC, 1], f32)
            nc.vector.reciprocal(out=drec, in_=dden)
            nc.scalar.activation(
                out=CHF[:, F_DIFF : F_DIFF + CTX], in_=dtmp, func=AF.Copy, scale=drec
            )

            # rolling std channel (mirrors reference cumsum formulation)
            ctx2 = fpool.tile([BC, CTX], f32)
            nc.vector.tensor_tensor(out=ctx2, in0=ctx, in1=ctx, op=OP.mult)
            cs = fpool.tile([BC, CTX], f32)
            nc.vector.tensor_tensor_scan(
                out=cs, data0=ctx, data1=ctx, initial=0.0, op0=OP.add, op1=OP.bypass
            )
            cs2 = fpool.tile([BC, CTX], f32)
            nc.vector.tensor_tensor_scan(
                out=cs2, data0=ctx2, data1=ctx2, initial=0.0, op0=OP.add, op1=OP.bypass
            )
            s1 = fpool.tile([BC, CTX], f32)
            s2 = fpool.tile([BC, CTX], f32)
            for (st, src) in ((s1, cs), (s2, cs2)):
                nc.gpsimd.tensor_copy(out=st[:, 0:5], in_=src[:, 0:5])
                nc.vector.tensor_tensor(
                    out=st[:, 5:CTX], in0=src[:, 5:CTX], in1=src[:, 0 : CTX - 5],
                    op=OP.subtract,
                )
            # t = s1*s1; var = (s2 - t/n) / (n-1); n: col2->3, col3->4, col>=4->5
            tsq = fpool.tile([BC, CTX], f32)
            nc.vector.tensor_tensor(out=tsq, in0=s1, in1=s1, op=OP.mult)
            var = fpool.tile([BC, CTX], f32)
            for lo, hi, ninv in ((2, 3, -1.0 / 3), (3, 4, -0.25), (4, CTX, -0.2)):
                nc.vector.scalar_tensor_tensor(
                    out=var[:, lo:hi], in0=tsq[:, lo:hi], scalar=ninv,
                    in1=s2[:, lo:hi], op0=OP.mult, op1=OP.add,
                )
            for lo, hi, dinv in ((2, 3, 0.5), (3, 4, 1.0 / 3), (4, CTX, 0.25)):
                nc.vector.tensor_scalar(
                    out=var[:, lo:hi], in0=var[:, lo:hi], scalar1=dinv, scalar2=1e-12,
                    op0=OP.mult, op1=OP.max,
                )
            rst_raw = fpool.tile([BC, CTX], f32)
            nc.gpsimd.memset(rst_raw[:, 0:2], 0.0)
            nc.scalar.activation(
                out=rst_raw[:, 2:CTX], in_=var[:, 2:CTX], func=AF.Sqrt
            )
            rabs = fpool.tile([BC, CTX], f32)
            rsum = fpool.tile([BC, 1], f32)
            nc.scalar.activation(out=rabs, in_=rst_raw, func=AF.Abs, accum_out=rsum)
            rden = fpool.tile([BC, 1], f32)
            nc.vector.tensor_scalar(
                out=rden, in0=rsum, scalar1=1.0 / CTX, scalar2=1e-6,
                op0=OP.mult, op1=OP.max,
            )
            rrec = fpool.tile([BC, 1], f32)
            nc.vector.reciprocal(out=rrec, in_=rden)
            nc.scalar.activation(
                out=CHF[:, F_RSTD : F_RSTD + CTX], in_=rst_raw, func=AF.Copy, scale=rrec
            )

            # time-feature channels: CHF[:, F_TF + 336*f + t] = ptfc[b, t*4 + f]
            for f in range(NTF):
                nc.gpsimd.tensor_copy(
                    out=CHF[:, F_TF + f * CTX : F_TF + (f + 1) * CTX],
                    in_=ptf_sb.rearrange("b (t f) -> b t f", f=NTF)[:, :, f],
                )
            # future-time-feature block: CHF[:, F_FT + 30*f + p] = ftf[b, p*4 + f]
            for f in range(NTF):
                nc.gpsimd.tensor_copy(
                    out=CHF[:, F_FT + f * PRED : F_FT + (f + 1) * PRED],
                    in_=ftf_sb.rearrange("b (p f) -> b p f", f=NTF)[:, :, f],
                )

            # ---------------- median via max8/match_replace cascade (DVE) ------
            mwork = fpool.tile([BC, CTX], f32)
            nc.vector.tensor_copy(out=mwork, in_=ctx)
            n_rounds = (CTX // 2) // 8  # 21 -> extracts top 168
            m8_prev = None
            for r in range(n_rounds):
                m8 = m8pool.tile([BC, 8], f32, tag="m8")
                nc.vector.max(out=m8, in_=mwork)
                nc.vector.match_replace(
                    out=mwork, in_to_replace=m8, in_values=mwork, imm_value=-1e30
                )
                m8_prev = m8
            m8_last = m8pool.tile([BC, 8], f32, tag="m8")
            nc.vector.max(out=m8_last, in_=mwork)
            a168 = m8_prev[:, 7:8]   # 168th largest == a[168] (0-idx ascending)
            a167 = m8_last[:, 0:1]   # 169th largest == a[167]
            # jnp.median linear interp: a167 + 0.5*(a168 - a167)
            dmed = fpool.tile([BC, 1], f32)
            nc.vector.tensor_tensor(out=dmed, in0=a168, in1=a167, op=OP.subtract)
            loc = fpool.tile([BC, 1], f32)
            nc.vector.scalar_tensor_tensor(
                out=loc, in0=dmed, scalar=0.5, in1=a167, op0=OP.mult, op1=OP.add
            )
            mloc = fpool.tile([BC, 1], f32)
            nc.vector.tensor_scalar(out=mloc, in0=loc, scalar1=-1.0, scalar2=None, op0=OP.mult)
            sabs = fpool.tile([BC, CTX], f32)
            ssum = fpool.tile([BC, 1], f32)
            nc.scalar.activation(
                out=sabs, in_=ctx, func=AF.Abs, bias=mloc, scale=1.0, accum_out=ssum
            )
            scale = fpool.tile([BC, 1], f32)
            nc.vector.tensor_scalar(
                out=scale, in0=ssum, scalar1=1.0 / CTX, scalar2=1e-6,
                op0=OP.mult, op1=OP.max,
            )
            rs = fpool.tile([BC, 1], f32)
            nc.vector.reciprocal(out=rs, in_=scale)

            # ---------------- transpose CHF -> xT, base matmuls ----------------
            # bf16 identity for bf16 transposes
            Ib = i8[:, 0:128]
            xT = cpool.tile([128, N_CHUNK * 128], bf16)
            nc.gpsimd.memset(xT[:, (N_CHUNK - 1) * 128 :], 0.0)
            with tc.tile_pool(name="tps", bufs=3, space="PSUM") as tpool:
                # 9 groups of 4 chunks share one psum bank + one ACT copy
                for j0 in range(0, 36, 4):
                    tps = tpool.tile([128, 512], bf16, tag="tr")
                    for j in range(j0, j0 + 4):
                        nc.tensor.transpose(
                            out=tps[:, (j - j0) * 128 : (j - j0 + 1) * 128],
                            in_=CHF[:, j * 128 : (j + 1) * 128], identity=Ib,
                        )
                    nc.scalar.copy(
                        out=xT[:, j0 * 128 : (j0 + 4) * 128], in_=tps
                    )
                # chunks 36 (full) and 37 (88 rows) individually
                for j in (36, 37):
                    F = 128 if j < N_CHUNK - 1 else nrem
                    tps1 = tpool.tile([128, 512], bf16, tag="tr")
                    nc.tensor.transpose(
                        out=tps1[:F, 0:128],
                        in_=CHF[:, j * 128 : j * 128 + F], identity=Ib,
                    )
                    nc.scalar.copy(
                        out=xT[:F, j * 128 : (j + 1) * 128], in_=tps1[:F, 0:128]
                    )

            with tc.tile_pool(name="bps", bufs=1, space="PSUM") as bpool:
                psum_lag = bpool.tile([128, 128], f32)
                psum_rest = bpool.tile([128, 128], f32)
                psum_q = bpool.tile([1, 128], f32)
                for j in range(LAG_CHUNKS):
                    sl = slice(j * 128, (j + 1) * 128)
                    nc.tensor.matmul(
                        psum_lag, lhsT=xT[:, sl], rhs=w1f[:, sl],
                        start=(j == 0), stop=(j == LAG_CHUNKS - 1),
                    )
                    nc.tensor.matmul(
                        psum_q, lhsT=ones_col, rhs=w1f[:, sl],
                        start=(j == 0), stop=(j == LAG_CHUNKS - 1),
                    )
                for j in range(LAG_CHUNKS, N_CHUNK):
                    sl = slice(j * 128, (j + 1) * 128)
                    nc.tensor.matmul(
                        psum_rest, lhsT=xT[:, sl], rhs=w1f[:, sl],
                        start=(j == LAG_CHUNKS), stop=False,
                    )
                nc.tensor.matmul(psum_rest, lhsT=ones2, rhs=w1th, start=False, stop=True)

                q_row = fpool.tile([1, 128], f32)
                nc.vector.tensor_copy(out=q_row, in_=psum_q)
                psum_qB = bpool.tile([128, 128], f32)
                nc.tensor.matmul(psum_qB, lhsT=ones_row, rhs=q_row, start=True, stop=True)

                mlr = fpool.tile([BC, 1], f32)
                nc.vector.tensor_scalar(
                    out=mlr, in0=loc, scalar1=rs, scalar2=-1.0, op0=OP.mult, op1=OP.mult
                )
                t1 = fpool.tile([128, 128], f32)
                nc.vector.tensor_scalar(out=t1, in0=psum_lag, scalar1=rs, scalar2=None, op0=OP.mult)
                t2 = fpool.tile([128, 128], f32)
                nc.vector.scalar_tensor_tensor(
                    out=t2, in0=psum_qB, scalar=mlr, in1=t1, op0=OP.mult, op1=OP.add
                )
                baseN = fpool.tile([128, 128], f32)
                nc.vector.tensor_tensor(out=baseN, in0=t2, in1=psum_rest, op=OP.add)
                basebf = cpool.tile([128, 128], bf16)
                nc.scalar.copy(out=basebf, in_=baseN)

            # ---------------- sampling loop ----------------
            gelu = AF.Gelu_apprx_tanh
            with (
                tc.tile_pool(name="zp", bufs=2) as zpool,
                tc.tile_pool(name="hp", bufs=2) as hpool,
                tc.tile_pool(name="ap", bufs=2) as apool,
                tc.tile_pool(name="op", bufs=2) as opool,
                tc.tile_pool(name="sp", bufs=1, space="PSUM") as spool,
            ):
                for g in range(N_GROUPS):
                    Gg = min(G, NS - g * G)
                    W = Gg * 128
                    P = Gg * PRED
                    zt_g = zpool.tile([PRED, G * 128], bf16, tag="zt")
                    nc.sync.dma_start(
                        out=zt_g[:, :W], in_=zt[:, g * G * 128 : g * G * 128 + W]
                    )
                    # A = z*scale + loc  (b3 handled by rank-1 inject into psum_u)
                    A_g = apool.tile([BC, G * PRED], f32, tag="A")
                    nc.vector.tensor_scalar(
                        out=A_g[:, :P],
                        in0=zn_sb[:, g * G * PRED : g * G * PRED + P],
                        scalar1=scale, scalar2=loc, op0=OP.mult, op1=OP.add,
                    )
                    psum1 = spool.tile([128, G * 128], f32, tag="p1", bufs=2)
                    for k in range(W // 512):
                        sl = slice(k * 512, (k + 1) * 512)
                        nc.tensor.matmul(
                            psum1[:, sl], lhsT=basebf, rhs=i8, start=True, stop=False,
                        )
                        nc.tensor.matmul(
                            psum1[:, sl], lhsT=w1zb, rhs=zt_g[:, sl],
                            start=False, stop=True,
                        )
                    h1T = hpool.tile([128, G * 128], bf16, tag="h1")
                    nc.scalar.activation(out=h1T[:, :W], in_=psum1[:, :W], func=gelu, bias=b1c)
                    psum2 = spool.tile([128, G * 128], f32, tag="p2", bufs=2)
                    for k in range(W // 512):
                        sl = slice(k * 512, (k + 1) * 512)
                        nc.tensor.matmul(
                            psum2[:, sl], lhsT=w2b, rhs=h1T[:, sl], start=True, stop=True,
                            skip_group_check=True,
                        )
                    h2T = hpool.tile([128, G * 128], f32, tag="h2")
                    nc.scalar.activation(out=h2T[:, :W], in_=psum2[:, :W], func=gelu, bias=b2c)
                    # L3 reuses psum2's bank (its L2 contents are dead after gelu2)
                    psum_u = psum2[:, 0 : G * PRED]
                    # rank-1 inject of -b3 across all sample blocks, then -h2@W3
                    nc.tensor.matmul(
                        psum_u[:, :240], lhsT=ones_row, rhs=mb3r8[:, :240],
                        start=True, stop=False, skip_group_check=True,
                    )
                    for gg in range(Gg):
                        nc.tensor.matmul(
                            psum_u[:, gg * PRED : (gg + 1) * PRED],
                            lhsT=h2T[:, gg * 128 : (gg + 1) * 128],
                            rhs=w3n, start=False, stop=True, skip_group_check=True,
                        )
                    ob = opool.tile([BC, G * PRED], f32, tag="ob")
                    nc.vector.scalar_tensor_tensor(
                        out=ob[:, :P], in0=psum_u[:, :P], scalar=scale, in1=A_g[:, :P],
                        op0=OP.mult, op1=OP.add,
                    )
                    nc.sync.dma_start(
                        out=out[:, g * G * PRED : g * G * PRED + P], in_=ob[:, :P]
                    )

    nc.compile()
    _cache["nc"] = nc
    return nc


def _in_maps(inputs):
    z = _get_z()  # [NS, B, PRED]
    pt_full = np.ascontiguousarray(inputs["past_target"], np.float32)
    ptf_full = np.ascontiguousarray(inputs["past_time_feat"], np.float32)
    ftf_full = np.ascontiguousarray(inputs["future_time_feat"], np.float32)
    w1 = np.ascontiguousarray(inputs["W1"], np.float32)
    w2 = np.ascontiguousarray(inputs["W2"], np.float32)
    w3 = np.ascontiguousarray(inputs["W3"], np.float32)
    b1 = np.ascontiguousarray(inputs["b1"], np.float32)
    b2 = np.ascontiguousarray(inputs["b2"], np.float32)
    b3 = np.ascontiguousarray(inputs["b3"], np.float32)

    maps = []
    for i in range(NCORES):
        sl = slice(i * BC, (i + 1) * BC)
        import ml_dtypes

        zc = z[:, sl, :]  # [NS, BC, PRED]
        # zt[d, s*128+b] = z[s,b,d]  (bf16 for the full-rate W1z matmul)
        ztc = np.ascontiguousarray(
            zc.transpose(2, 0, 1).reshape(PRED, NS * BC)
        ).astype(ml_dtypes.bfloat16)
        znc = np.ascontiguousarray(
            zc.transpose(1, 0, 2).reshape(BC, NS * PRED), np.float32
        )
        maps.append(
            {
                "pt": pt_full[sl],
                "ptfc": np.ascontiguousarray(
                    ptf_full[sl, T0:TP, :].reshape(BC, CTX * NTF)
                ),
                "ftf": np.ascontiguousarray(ftf_full[sl].reshape(BC, PRED * NTF)),
                "w1": w1, "w2": w2, "w3": w3, "b1": b1, "b2": b2, "b3": b3,
                "zt": ztc, "zn": znc,
            }
        )
    return maps


def kernel(**inputs):
    nc = _build()
    from concourse.bass_utils import run_bass_kernel_spmd

    res = run_bass_kernel_spmd(nc, _in_maps(inputs), core_ids=list(range(NCORES)))
    _cache["last_result"] = res
    out = np.concatenate(
        [r["out"].reshape(BC, NS, PRED) for r in res.results], axis=0
    )
    return out


if __name__ == "__main__":
    _build()
    print("build ok")
